# revision 55
# baseline (speedup 1.0000x reference)
"""Trainium2 Bass kernel for nn_Model_20925080666713 (4-layer dense transformer).

Model (per reference): B=32, S=512, D=512, H=8, L=4, FFN=1024, fp32.
  out = x + pe
  per layer: Q,K,V = out@W* + b*; "raw view" attention over (B*H, S, DH)
  contiguous reshape; a = LN1(ctx@Wo + bo + out); out = LN2(relu(a@W1+b1)@W2 + b2 + a)

Sharding: pure data-parallel over batch across 8 NeuronCores (4 batch elems,
i.e. 2048 tokens, per core). Zero collectives. Weights replicated.

Key observation about the "faithful raw view": Q.reshape(B*H,S,DH) of the
contiguous (B,S,D) tensor makes attention BLOCK-LOCAL: slice (b,h) is the
contiguous 64-token x 512-channel block Q[b, 64h:64h+64, :] reinterpreted as
(512, 64) with row q = sm*8+dg (sm = s%64, dg = d//64) and col e = d%64.
So per 64-token block: att[q,kq] = sum_e Q[tb+sm, dg*64+e] K[tb+sm', dg'*64+e].

Device layout strategy (per core, all matmuls bf16, accum fp32):
 - residual stream token-major [128t x (16,512)] for LayerNorm (free-dim stats)
 - PE-transposed feature-major slices [128d x (4,512)] feed projections
 - Q,K projections emitted feature-major; logits per block are 4 col-packed
   K=64 matmul pairs into [128,1024] double-bank PSUM tiles; exp runs as 2
   wide ACT calls (amortizes the 352-cycle ACT ramp)
 - V stored stacked [128=(dg' parity, sm'), tcw, half, m, 128]: ctx is 4
   single K=128 matmuls per block; cols 64:128 of the stacked V are ALL ones,
   so PSUM rows 64:128 accumulate the softmax denominators replicated across
   64 partitions -- the matmul itself performs the broadcast
 - softmax denominators: DVE fast-reciprocal of PSUM rows 64:128, then
   multiply-during-evacuation (no broadcast step at all)
 - LayerNorm rsqrt computed DVE-only (exact reciprocal + sqrt bit-trick
   seed + 2 Newton steps, batched 4 chunks/slice) so ACT only ever runs
   Exp/Copy/Relu from one table set -> no ACT_TABLE_LOAD thrash
 - all partition-crossing rearranges are SBUF->SBUF DMAs (block moves only)

The fast path assumes bv=bo=b2=0, ln*_g=1, ln*_b=0 (true for this problem's
setup_inputs); kernel() verifies at runtime and falls back to exact numpy
otherwise. bq, bk, b1 are applied on-device (free via ACT bias).
"""
import sys
if "/opt/trn_rl_repo" not in sys.path:
    sys.path.insert(0, "/opt/trn_rl_repo")

import numpy as np
import ml_dtypes

B, S, D, H, L, FFN = 32, 512, 512, 8, 4, 1024
DH = D // H
EPS = 1e-5
NCORES = 8
BL = B // NCORES          # batch per core
T = BL * S                # tokens per core = 2048
NCHUNK = T // 128         # 16 token chunks of 128
NSLICE = T // 512         # 4 token slices of 512
F32 = None  # set after imports
BF16 = None

_PROG_CACHE = {}


def _build_program(n_layers=L):
    import concourse.bass as bass
    import concourse.mybir as mybir
    import concourse.tile as tile
    from concourse import bacc
    from concourse.masks import make_identity

    f32 = mybir.dt.float32
    bf16 = mybir.dt.bfloat16
    AF = mybir.ActivationFunctionType

    nc = bacc.Bacc("TRN2", target_bir_lowering=False, debug=False,
                   num_devices=NCORES)

    # ---- DRAM parameters (per-core shard of x / out; weights replicated) ----
    x_d = nc.dram_tensor("x", [BL, S, D], f32, kind="ExternalInput").ap()
    pe_d = nc.dram_tensor("pe", [S, D], f32, kind="ExternalInput").ap()
    wq_d = nc.dram_tensor("wq", [L, D, D], bf16, kind="ExternalInput").ap()
    wk_d = nc.dram_tensor("wk", [L, D, D], bf16, kind="ExternalInput").ap()
    wv_d = nc.dram_tensor("wv", [L, D, D], bf16, kind="ExternalInput").ap()
    wo_d = nc.dram_tensor("wo", [L, D, D], bf16, kind="ExternalInput").ap()
    w1_d = nc.dram_tensor("w1", [L, D, FFN], bf16, kind="ExternalInput").ap()
    w2_d = nc.dram_tensor("w2", [L, FFN, D], bf16, kind="ExternalInput").ap()
    bq_d = nc.dram_tensor("bq", [L, D], f32, kind="ExternalInput").ap()
    bk_d = nc.dram_tensor("bk", [L, D], f32, kind="ExternalInput").ap()
    b1_d = nc.dram_tensor("b1", [L, FFN], f32, kind="ExternalInput").ap()
    out_d = nc.dram_tensor("out", [BL, S * D], f32, kind="ExternalOutput").ap()
    ov = out_d.rearrange("b (s d) -> b s d", d=D)

    with tile.TileContext(nc) as tc:
        _emit(nc, tc, tile, mybir, make_identity, n_layers,
              x_d, pe_d, wq_d, wk_d, wv_d, wo_d, w1_d, w2_d,
              bq_d, bk_d, b1_d, ov)
    nc.finalize()
    return nc


def _emit(nc, tc, tile, mybir, make_identity, n_layers,
          x_d, pe_d, wq_d, wk_d, wv_d, wo_d, w1_d, w2_d, bq_d, bk_d, b1_d, ov):
    from contextlib import ExitStack
    import concourse.bass as bass

    f32 = mybir.dt.float32
    bf16 = mybir.dt.bfloat16
    AF = mybir.ActivationFunctionType
    OP = mybir.AluOpType

    ctx = ExitStack()
    with ctx:
        # ---------------- pools ----------------
        consts = ctx.enter_context(tc.tile_pool(name="consts", bufs=1))
        stream = ctx.enter_context(tc.tile_pool(name="stream", bufs=2))
        streamT = ctx.enter_context(tc.tile_pool(name="streamT", bufs=3))
        wq_p = ctx.enter_context(tc.tile_pool(name="wq_p", bufs=2))
        wk_p = ctx.enter_context(tc.tile_pool(name="wk_p", bufs=2))
        wv_p = ctx.enter_context(tc.tile_pool(name="wv_p", bufs=1))
        wo_p = ctx.enter_context(tc.tile_pool(name="wo_p", bufs=1))
        w1_p = ctx.enter_context(tc.tile_pool(name="w1_p", bufs=1))
        w2_p = ctx.enter_context(tc.tile_pool(name="w2_p", bufs=1))
        qt_p = ctx.enter_context(tc.tile_pool(name="qt_p", bufs=2))
        kt_p = ctx.enter_context(tc.tile_pool(name="kt_p", bufs=2))
        vtmp_p = ctx.enter_context(tc.tile_pool(name="vtmp_p", bufs=2))
        vstack_p = ctx.enter_context(tc.tile_pool(name="vstack_p", bufs=2))
        qhT_p = ctx.enter_context(tc.tile_pool(name="qhT_p", bufs=2))
        ktsw_p = ctx.enter_context(tc.tile_pool(name="ktsw_p", bufs=2))
        recip_p = ctx.enter_context(tc.tile_pool(name="recip_p", bufs=2))
        attexp_p = ctx.enter_context(tc.tile_pool(name="attexp_p", bufs=4))
        ctxsb_p = ctx.enter_context(tc.tile_pool(name="ctxsb_p", bufs=4))
        ctxt_p = ctx.enter_context(tc.tile_pool(name="ctxt_p", bufs=6))
        ht_p = ctx.enter_context(tc.tile_pool(name="ht_p", bufs=2))
        lnin_p = ctx.enter_context(tc.tile_pool(name="lnin_p", bufs=5))
        stats_p = ctx.enter_context(tc.tile_pool(name="stats_p", bufs=8))
        rs_p = ctx.enter_context(tc.tile_pool(name="rs_p", bufs=14))
        xin_p = ctx.enter_context(tc.tile_pool(name="xin_p", bufs=2))
        outst_p = ctx.enter_context(tc.tile_pool(name="outst_p", bufs=3))
        ps_p = ctx.enter_context(tc.tile_pool(name="ps_p", bufs=2, space="PSUM"))
        attps_p = ctx.enter_context(tc.tile_pool(name="attps_p", bufs=2, space="PSUM"))
        ctxps_p = ctx.enter_context(tc.tile_pool(name="ctxps_p", bufs=2, space="PSUM"))

        # ---------------- constants ----------------
        ident = consts.tile([128, 128], bf16, tag="ident")
        make_identity(nc, ident)
        pe_sb = consts.tile([128, 4, D], bf16, tag="pe_sb")
        for sc in range(4):
            pe_st = xin_p.tile([128, 512], f32, tag="xin", name=f"pe_st{sc}")
            nc.sync.dma_start(out=pe_st, in_=pe_d[sc * 128:sc * 128 + 128, :])
            nc.vector.tensor_copy(pe_sb[:, sc, :], pe_st)
        bq_sb = consts.tile([128, L, 4], f32, tag="bq_sb")
        nc.sync.dma_start(out=bq_sb, in_=bq_d.rearrange("l (a p) -> p l a", p=128))
        bk_sb = consts.tile([128, L, 4], f32, tag="bk_sb")
        nc.sync.dma_start(out=bk_sb, in_=bk_d.rearrange("l (a p) -> p l a", p=128))
        b1_sb = consts.tile([128, L, 8], f32, tag="b1_sb")
        nc.sync.dma_start(out=b1_sb, in_=b1_d.rearrange("l (a p) -> p l a", p=128))
        ones_c = consts.tile([128, 64], bf16, tag="ones_c")
        nc.vector.memset(ones_c, 1.0)
        # two persistent stacked-V buffers, alternated across slices; the
        # ones half (cols 0:64 of each group) is filled ONCE here instead
        # of re-filling 512KB per slice (V DMAs only ever touch 64:128)
        vst_bufs = []
        for vi in range(2):
            vt = vstack_p.tile([128, 4, 2, 4, 128], bf16, tag="vstack",
                               name=f"vstack{vi}")
            vt3 = vt.rearrange("p a h m c -> p (a h m) c")
            nc.sync.dma_start(out=vt3[:, :, 0:64],
                              in_=ones_c[:, None, :].to_broadcast((128, 32, 64)))
            vst_bufs.append(vt)



        def transpose_slice(src, ts, nm):
            """token-major slice [128, 4 chunks, 512] -> feature-major
            [128, 4 dj, 512 tok]."""
            dst = streamT.tile([128, 4, 512], bf16, tag="streamT",
                               name=f"{nm}{ts}")
            for dj in range(4):
                ps = ps_p.tile([128, 512], bf16, tag="ps")
                for k in range(4):
                    tcn = ts * 4 + k
                    nc.tensor.transpose(
                        ps[:, k * 128:(k + 1) * 128],
                        src[:, tcn, dj * 128:(dj + 1) * 128], ident)
                nc.scalar.activation(dst[:, dj, :], ps, AF.Copy)
            return dst

        u32 = mybir.dt.uint32

        def ln_begin():
            """Batched-per-slice LayerNorm state (4 chunks)."""
            mv4 = stats_p.tile([128, 4, 2], f32, tag="mv4")
            return {"mv4": mv4, "lns": [], "outs": []}

        def ln_chunk(st, tcw, ps_in, res_ap, out_ap):
            ln = lnin_p.tile([128, 512], f32, tag="lnin")
            nc.vector.tensor_add(ln, ps_in, res_ap)
            st6 = stats_p.tile([128, 6], f32, tag="st6")
            nc.vector.bn_stats(st6, ln)
            nc.vector.bn_aggr(st["mv4"][:, tcw, :], st6)
            st["lns"].append(ln)
            st["outs"].append(out_ap)

        def ln_finish(st):
            """rsqrt(var+eps) for 4 chunks entirely on DVE (no ACT sqrt ->
            no table switches): exact HW reciprocal, sqrt bit-trick seed
            (forward shift+add), 2 Newton steps using v as the known 1/r.
            Then the per-chunk (x-mu)*sd applies."""
            mv4 = st["mv4"]
            v4 = rs_p.tile([128, 4], f32, tag="v4")
            nc.vector.tensor_scalar(out=v4, in0=mv4[:, :, 1], scalar1=EPS,
                                    scalar2=None, op0=OP.add)
            r4 = rs_p.tile([128, 4], f32, tag="r4")
            nc.vector.reciprocal(r4, v4)
            h4 = rs_p.tile([128, 4], u32, tag="h4")
            nc.vector.tensor_scalar(out=h4, in0=r4.bitcast(u32), scalar1=1,
                                    scalar2=None, op0=OP.logical_shift_right)
            j4 = rs_p.tile([128, 4], u32, tag="j4")
            nc.vector.tensor_scalar(out=j4, in0=h4, scalar1=0x1FBD1DF5,
                                    scalar2=None, op0=OP.add)
            y = j4.bitcast(f32)
            for _ in range(2):
                t4 = rs_p.tile([128, 4], f32, tag="t4")
                nc.vector.tensor_mul(t4, y, y)
                nc.vector.tensor_mul(t4, t4, v4)
                nc.vector.tensor_scalar(out=t4, in0=t4, scalar1=-0.5,
                                        scalar2=1.5, op0=OP.mult, op1=OP.add)
                yn = rs_p.tile([128, 4], f32, tag="yn")
                nc.vector.tensor_mul(yn, y, t4)
                y = yn
            for c, (ln, out_ap) in enumerate(zip(st["lns"], st["outs"])):
                nc.vector.tensor_scalar(out=out_ap, in0=ln,
                                        scalar1=mv4[:, c, 0:1],
                                        scalar2=y[:, c:c + 1],
                                        op0=OP.subtract, op1=OP.mult)

        # ---------------- prologue: R0 = x + pe ----------------
        R = stream.tile([128, NCHUNK, 512], bf16, tag="stream")
        for tcn in range(NCHUNK):
            xt = xin_p.tile([128, 512], f32, tag="xin")
            nc.sync.dma_start(out=xt, in_=x_d[tcn // 4,
                                             (tcn % 4) * 128:(tcn % 4) * 128 + 128, :])
            nc.vector.tensor_add(R[:, tcn, :], xt, pe_sb[:, tcn % 4, :])

        # ---------------- layers ----------------
        def load_weights(l):
            wq_t = wq_p.tile([128, 4, D], bf16, tag="wq")
            wk_t = wk_p.tile([128, 4, D], bf16, tag="wk")
            wv_t = wv_p.tile([128, 4, D], bf16, tag="wv")
            wo_t = wo_p.tile([128, 4, D], bf16, tag="wo")
            w1_t = w1_p.tile([128, 4, FFN], bf16, tag="w1")
            w2_t = w2_p.tile([128, 8, D], bf16, tag="w2")
            for dk in range(4):
                nc.sync.dma_start(out=wq_t[:, dk, :], in_=wq_d[l, dk * 128:dk * 128 + 128, :])
                nc.sync.dma_start(out=wk_t[:, dk, :], in_=wk_d[l, dk * 128:dk * 128 + 128, :])
                nc.sync.dma_start(out=wv_t[:, dk, :], in_=wv_d[l, dk * 128:dk * 128 + 128, :])
                nc.sync.dma_start(out=wo_t[:, dk, :], in_=wo_d[l, dk * 128:dk * 128 + 128, :])
                nc.sync.dma_start(out=w1_t[:, dk, :], in_=w1_d[l, dk * 128:dk * 128 + 128, :])
            for fk in range(8):
                nc.sync.dma_start(out=w2_t[:, fk, :], in_=w2_d[l, fk * 128:fk * 128 + 128, :])
            return (wq_t, wk_t, wv_t, wo_t, w1_t, w2_t)

        def emit_qkv(l, ts, Rcur, wq_t, wk_t, wv_t):
            """transpose + Q/K/V projections + rearrange DMAs for one
            slice of layer l (Rcur = that layer's input stream)."""
            rt = transpose_slice(Rcur, ts, f"rt{l}")
            qt_t = qt_p.tile([128, 4, 512], bf16, tag="qt", name=f"qt{l}{ts}")
            kt_t = kt_p.tile([128, 4, 512], bf16, tag="kt", name=f"kt{l}{ts}")
            for (w_t, b_sb, dst) in ((wq_t, bq_sb, qt_t), (wk_t, bk_sb, kt_t)):
                for dc in range(4):
                    ps = ps_p.tile([128, 512], f32, tag="ps", name=f"ps{l}{ts}{dc}")
                    for dk in range(4):
                        nc.tensor.matmul(ps, w_t[:, dk, dc * 128:dc * 128 + 128],
                                         rt[:, dk, :],
                                         start=(dk == 0), stop=(dk == 3))
                    nc.scalar.activation(dst[:, dc, :], ps, AF.Identity,
                                         bias=b_sb[:, l, dc:dc + 1].opt())
            kt_sw = ktsw_p.tile([64, 4, 512], bf16, tag="ktsw",
                                name=f"ktsw{l}{ts}")
            nc.sync.dma_start(out=kt_sw[0:64, :, :], in_=kt_t[64:128, :, :])
            qd_sl = qhT_p.tile([64, 8, 512], bf16, tag="qhT", name=f"qd{l}{ts}")
            qd_v = qd_sl.rearrange("p b (a c) -> p b a c", a=4)
            qt_v = qt_t.rearrange("p a (b c) -> p b a c", b=8)
            for dt4 in range(4):
                nc.sync.dma_start(out=qd_v[0:64, :, dt4, 0:64],
                                  in_=qt_v[0:64, :, dt4, :])
                nc.sync.dma_start(out=qd_v[0:64, :, dt4, 64:128],
                                  in_=qt_v[64:128, :, dt4, :])
            # stacked V [128, tcw, half, m, 128]: cols 0:64 are ALL ones
            # (persistent, filled once at startup), cols 64:128 hold V
            # values (even-dg' rows 0:64, odd rows 64:128).  The ctx
            # matmul thus replicates the softmax denominators into PSUM
            # rows 0:64 (base partition 0, which the custom DVE
            # reciprocal requires) and ctx^T into rows 64:128.
            vstack = vst_bufs[(l * NSLICE + ts) % 2]
            for tcw in range(4):
                ps = ps_p.tile([128, 512], f32, tag="ps", name=f"psv{l}{ts}{tcw}")
                for dk in range(4):
                    nc.tensor.matmul(ps, rt[:, dk, tcw * 128:tcw * 128 + 128],
                                     wv_t[:, dk, :], start=(dk == 0), stop=(dk == 3))
                vtmp = vtmp_p.tile([128, 512], bf16, tag="vtmp",
                                   name=f"vtmp{l}{ts}{tcw}")
                nc.scalar.activation(vtmp, ps, AF.Copy)
                v5 = vtmp.rearrange("p (m par e) -> p m par e", par=2, e=64)
                nc.sync.dma_start(out=vstack[0:64, tcw, 0, :, 64:128],
                                  in_=v5[0:64, :, 0, :])
                nc.sync.dma_start(out=vstack[64:128, tcw, 0, :, 64:128],
                                  in_=v5[0:64, :, 1, :])
                nc.sync.dma_start(out=vstack[0:64, tcw, 1, :, 64:128],
                                  in_=v5[64:128, :, 0, :])
                nc.sync.dma_start(out=vstack[64:128, tcw, 1, :, 64:128],
                                  in_=v5[64:128, :, 1, :])
            return kt_t, kt_sw, qd_sl, vstack

        weights = load_weights(0)
        next_slice_ops = emit_qkv(0, 0, R, weights[0], weights[1], weights[2])
        for l in range(n_layers):
            wq_t, wk_t, wv_t, wo_t, w1_t, w2_t = weights
            A = stream.tile([128, NCHUNK, 512], bf16, tag="stream")
            slice_ops = next_slice_ops
            for ts in range(NSLICE):
                kt_t, kt_sw, qd_sl, vstack = slice_ops

                # -- attention: 8 blocks of 64 tokens; each chunk's Wo
                # projection + LN stats are emitted as soon as the chunk's
                # second half-block lands (denser PE stream, fewer HAM
                # throttle windows) --
                lnst = ln_begin()
                ctx_ch = []
                for blk in range(8):
                    tb = blk * 64
                    tcw, half = blk // 2, blk % 2
                    # logits: 4 col-packed pairs into 2 double-bank PSUM
                    # tiles so exp runs as 2 wide ACT calls; ctx: 4 K=128
                    # matmuls against the stacked V (even-dg' rows 0:64,
                    # odd rows 64:128, ones col 64 accumulates softmax
                    # denominators into cps row 64 for free).
                    axs = []
                    for p2 in range(2):
                        aps2 = attps_p.tile([128, 1024], f32, tag="attps")
                        for mm in range(2):
                            m = 2 * p2 + mm
                            c = mm * 512
                            nc.tensor.matmul(aps2[0:64, c:c + 512],
                                             kt_t[0:64, m, tb:tb + 64],
                                             qd_sl[0:64, blk, :],
                                             start=True, stop=True)
                            nc.tensor.matmul(aps2[64:128, c:c + 512],
                                             kt_sw[0:64, m, tb:tb + 64],
                                             qd_sl[0:64, blk, :],
                                             start=True, stop=True)
                        ax2 = attexp_p.tile([128, 1024], bf16, tag="attexp")
                        nc.scalar.activation(ax2, aps2, AF.Exp,
                                             scale=float(DH ** -0.5))
                        axs.append(ax2)
                    cps = ctxps_p.tile([128, 512], f32, tag="ctxps")
                    for m in range(4):
                        c = (m % 2) * 512
                        nc.tensor.matmul(cps,
                                         vstack[:, tcw, half, m, :],
                                         axs[m // 2][:, c:c + 512],
                                         start=(m == 0), stop=(m == 3))
                    # rows 0:64 of cps all hold the softmax denominators
                    # (the matmul broadcast them); fast-reciprocal those and
                    # multiply-evacuate to bf16 -- no separate broadcast step
                    rcf = recip_p.tile([64, 512], f32, tag="recip")
                    nc.vector.reciprocal_approx_fast(out=rcf, in_=cps[0:64, :])
                    csb = ctxsb_p.tile([72, 512], bf16, tag="ctxsb")
                    nc.vector.tensor_mul(csb[0:64, :], cps[64:128, :], rcf)
                    csb_v = csb.rearrange("p (a c) -> p a c", a=4)
                    # ctx goes to per-chunk feature-major tiles (finer deps for
                    # the Wo matmuls)
                    if half == 0:
                        ctxc = ctxt_p.tile([128, 4, 128], bf16, tag="ctxt")
                        ctx_ch.append(ctxc)
                    c0 = half * 64
                    nc.sync.dma_start(out=ctxc[0:64, :, c0:c0 + 64],
                                      in_=csb_v[0:64, :, 0:64])
                    nc.sync.dma_start(out=ctxc[64:128, :, c0:c0 + 64],
                                      in_=csb_v[0:64, :, 64:128])

                    if half == 1:
                        # Wo projection + residual + LN1 stats for this chunk
                        tcn = ts * 4 + tcw
                        ps = ps_p.tile([128, 512], f32, tag="ps")
                        for dk in range(4):
                            nc.tensor.matmul(ps, ctx_ch[tcw][:, dk, :],
                                             wo_t[:, dk, :],
                                             start=(dk == 0), stop=(dk == 3))
                        ln_chunk(lnst, tcw, ps, R[:, tcn, :], A[:, tcn, :])

                    if blk == 3 and ts + 1 < NSLICE:
                        slice_ops = emit_qkv(l, ts + 1, R, wq_t, wk_t, wv_t)
                ln_finish(lnst)

            # ---------------- FFN ----------------
            if l == n_layers - 1:
                R_next = None
            else:
                R_next = stream.tile([128, NCHUNK, 512], bf16, tag="stream")
            for ts in range(NSLICE):
                at_s = transpose_slice(A, ts, "at")
                ht_sl = ht_p.tile([128, 8, 512], bf16, tag="ht")
                for fc in range(8):
                    ps = ps_p.tile([128, 512], f32, tag="ps")
                    for dk in range(4):
                        nc.tensor.matmul(ps, w1_t[:, dk, fc * 128:fc * 128 + 128],
                                         at_s[:, dk, :],
                                         start=(dk == 0), stop=(dk == 3))
                    # bias+relu fused on DVE -- keeps ACT's strict FIFO free
                    # for the attention exps and cross-layer Q/K evacuations
                    nc.vector.tensor_scalar(
                        out=ht_sl[:, fc, :], in0=ps,
                        scalar1=b1_sb[:, l, fc:fc + 1].opt(), scalar2=0.0,
                        op0=OP.add, op1=OP.max)
                lnst = ln_begin()
                ots = []
                for tcw in range(4):
                    tcn = ts * 4 + tcw
                    ps = ps_p.tile([128, 512], f32, tag="ps")
                    for fk in range(8):
                        nc.tensor.matmul(ps, ht_sl[:, fk, tcw * 128:tcw * 128 + 128],
                                         w2_t[:, fk, :], start=(fk == 0), stop=(fk == 7))
                    if R_next is None:
                        ot = outst_p.tile([128, 512], f32, tag="outst")
                        ln_chunk(lnst, tcw, ps, A[:, tcn, :], ot)
                        ots.append((tcn, ot))
                    else:
                        ln_chunk(lnst, tcw, ps, A[:, tcn, :], R_next[:, tcn, :])
                ln_finish(lnst)
                for tcn, ot in ots:
                    b = tcn // 4
                    s0 = (tcn % 4) * 128
                    nc.sync.dma_start(out=ov[b, s0:s0 + 128, :], in_=ot)
                # cross-layer software pipeline: prefetch next layer's
                # weights mid-FFN, and emit its first transpose+QKV right
                # after slice 2 so the PE stream never drains at the
                # layer boundary (slice 0 of R_next is complete by then)
                if l + 1 < n_layers:
                    if ts == 1:
                        weights_next = load_weights(l + 1)
                    elif ts == 2:
                        next_slice_ops = emit_qkv(
                            l + 1, 0, R_next,
                            weights_next[0], weights_next[1], weights_next[2])
            if l + 1 < n_layers:
                weights = weights_next
            R = R_next


# ---------------------------------------------------------------------------
# host side
# ---------------------------------------------------------------------------

def _numpy_reference(x, pe, Wq, bq, Wk, bk, Wv, bv, Wo, bo, ln1_g, ln1_b,
                     W1, b1, W2, b2, ln2_g, ln2_b):
    """Exact fp64->fp32 fallback, mirrors reference.py (used only if the
    fast-path constant assumptions do not hold)."""
    def ln(x_, g, b_):
        mu = x_.mean(-1, keepdims=True)
        var = ((x_ - mu) ** 2).mean(-1, keepdims=True)
        return (x_ - mu) / np.sqrt(var + EPS) * g + b_
    out = x.astype(np.float64) + pe.astype(np.float64)
    scale = DH ** -0.5
    for l in range(L):
        Q = out @ Wq[l].astype(np.float64) + bq[l]
        K = out @ Wk[l].astype(np.float64) + bk[l]
        V = out @ Wv[l].astype(np.float64) + bv[l]
        Qh = Q.reshape(B * H, S, DH)
        Kh = K.reshape(B * H, S, DH)
        Vh = V.reshape(B * H, S, DH)
        att = np.einsum("bqd,bkd->bqk", Qh, Kh) * scale
        att = att - att.max(-1, keepdims=True)
        att = np.exp(att)
        att /= att.sum(-1, keepdims=True)
        ctxv = np.einsum("bqk,bkd->bqd", att, Vh).reshape(B, S, D)
        a = ln(ctxv @ Wo[l].astype(np.float64) + bo[l] + out, ln1_g[l], ln1_b[l])
        h = np.maximum(a @ W1[l].astype(np.float64) + b1[l], 0.0)
        out = ln(h @ W2[l].astype(np.float64) + b2[l] + a, ln2_g[l], ln2_b[l])
    return out.reshape(B, S * D).astype(np.float32)


def _fast_path_ok(inputs):
    z = lambda a: np.all(np.asarray(a) == 0.0)
    o = lambda a: np.all(np.asarray(a) == 1.0)
    return (z(inputs["bv"]) and z(inputs["bo"]) and z(inputs["b2"])
            and o(inputs["ln1_g"]) and z(inputs["ln1_b"])
            and o(inputs["ln2_g"]) and z(inputs["ln2_b"]))


def kernel(**inputs):
    inputs = {k: np.asarray(v) for k, v in inputs.items()}
    if not _fast_path_ok(inputs):
        return _numpy_reference(**inputs)

    res = _run(inputs)
    return np.concatenate([res.results[i]["out"] for i in range(NCORES)], axis=0)


def _run(inputs, trace=False, **kw):
    from concourse.bass_utils import run_bass_kernel_spmd

    if "prog" not in _PROG_CACHE:
        _PROG_CACHE["prog"] = _build_program(L)
    nc = _PROG_CACHE["prog"]

    bf = ml_dtypes.bfloat16
    shared = {
        "pe": inputs["pe"].astype(np.float32),
        "wq": inputs["Wq"].astype(bf), "wk": inputs["Wk"].astype(bf),
        "wv": inputs["Wv"].astype(bf), "wo": inputs["Wo"].astype(bf),
        "w1": inputs["W1"].astype(bf), "w2": inputs["W2"].astype(bf),
        "bq": inputs["bq"].astype(np.float32),
        "bk": inputs["bk"].astype(np.float32),
        "b1": inputs["b1"].astype(np.float32),
    }
    x = inputs["x"].astype(np.float32)
    in_maps = [dict(shared, x=np.ascontiguousarray(x[i * BL:(i + 1) * BL]))
               for i in range(NCORES)]
    return run_bass_kernel_spmd(nc, in_maps, list(range(NCORES)),
                                trace=trace, **kw)


if __name__ == "__main__":
    import reference
    ins = {k: np.asarray(v) for k, v in reference.setup_inputs().items()}
    got = kernel(**ins)
    print("out shape:", got.shape, got.dtype)

